# revision 25
# baseline (speedup 1.0000x reference)
"""Trainium2 Bass kernel for the CoordinateDescent problem.

Problem: one Gauss-Seidel coordinate-descent sweep updating u then v for
rank-R factorization:  u' = GS(x @ v, v^T v), v' = GS(x^T @ u', u'^T u').
Shapes: x (4, 4096, 4096) f32, u/v (4, 4096, 16) f32.

Key transformation: the sequential R-step Gauss-Seidel sweep is linear in
(a, u_old) given the R x R Gram matrix B:
    u_new = (a + eps - u_old @ tril(B,-1)) @ inv(diag(B)+eps + triu(B,1))
so with host-precomputed (R x R, float64) coefficients the device only does
large matmuls:
    u_new = x @ (v @ W1) - u_old @ W3 + c
The v update needs B_v = u_new^T u_new and a_v = x^T u_new, whose shard
partials the device computes in the same single pass over x.

Sharding: 8 cores = (batch b = c//2) x (M-half h = c%2). Each core reads its
(2048, 4096) x-shard from HBM exactly once. a_v/b_v partials are reduced
across the 2-core pair on host (256KB), which also assembles the final
outputs (full-I/O contract).

All small tensors cross HBM in partition-contiguous packed layouts (host
repacks for free) so every DMA descriptor is >= 1KB; x streaming is
double-buffered 8 deep so the DMA engines never stall on compute.
"""

import numpy as np

from concourse import bacc, tile
import concourse.mybir as mybir
from concourse.bass_utils import run_bass_kernel_spmd

B, M, N, R = 4, 4096, 4096, 16
EPS = 1e-8
NCORES = 8
P = 128
MS = M // 2          # rows of x per core (2048)
MT = MS // P         # m-tiles per core (16)
NB = N // P          # n-blocks (32)
NS = N // 2          # v rows per core (2048)
NT = NS // P         # n-tiles per core for launch 2 (16)

F32 = mybir.dt.float32
F32R = mybir.dt.float32r   # same 4-byte data; PE transpose runs 1.5 vs 2.0
                           # cycles/row (values pass through unchanged)

_cache = {}


def _build_launch1(repeat=1, xbufs=8):
    nc = bacc.Bacc("TRN2", target_bir_lowering=False, debug=False,
                   num_devices=NCORES)

    xs_d = nc.dram_tensor("xs", [MS, N], F32, kind="ExternalInput")
    vw_d = nc.dram_tensor("vw", [P, NB * R], F32, kind="ExternalInput")
    # u_old^T with a ones row appended, transposed on host: [R+1, MS]
    ut_d = nc.dram_tensor("ust", [R + 1, MS], F32, kind="ExternalInput")
    wa_d = nc.dram_tensor("waug", [R + 1, R], F32, kind="ExternalInput")
    id_d = nc.dram_tensor("ident", [P, P], F32, kind="ExternalInput")
    uo_d = nc.dram_tensor("u_out", [MS, R], F32, kind="ExternalOutput")
    av_d = nc.dram_tensor("av_out", [P, NB * R], F32, kind="ExternalOutput")
    bv_d = nc.dram_tensor("bv_out", [R, R], F32, kind="ExternalOutput")

    xs_r = xs_d[:].rearrange("(t p) n -> t p n", p=P)       # [MT, P, N]
    uo_r = uo_d[:].rearrange("(t p) r -> t p r", p=P)       # [MT, P, R]
    vw_r = vw_d[:].rearrange("p (nb r) -> p nb r", r=R)     # [P, NB, R]
    av_r = av_d[:].rearrange("p (nb r) -> p nb r", r=R)     # [P, NB, R]

    with tile.TileContext(nc) as tc:
        with (
            tc.tile_pool(name="const", bufs=1) as cpool,
            tc.tile_pool(name="xin", bufs=xbufs) as xpool,
            tc.tile_pool(name="xtr", bufs=6) as xtpool,
            tc.tile_pool(name="small", bufs=3) as spool,
            tc.tile_pool(name="ps", bufs=2, space="PSUM") as pspool,
            tc.tile_pool(name="ps3", bufs=3, space="PSUM") as ps3pool,
            tc.tile_pool(name="acc", bufs=1, space="PSUM") as accpool,
        ):
            id_sb = cpool.tile([P, P], F32)
            nc.scalar.dma_start(id_sb[:], id_d[:])
            vw_sb = cpool.tile([P, NB, R], F32)
            nc.scalar.dma_start(vw_sb[:], vw_r)
            uaug = cpool.tile([R + 1, MS], F32)
            nc.scalar.dma_start(uaug[:], ut_d[:])
            wa_sb = cpool.tile([R + 1, R], F32)
            nc.scalar.dma_start(wa_sb[:], wa_d[:])

            bv_ps = accpool.tile([R, R], F32)
            av_acc = cpool.tile([P, NB, R], F32)    # SBUF accumulator

            GRP = 4                      # transposes batched per PSUM bank
            NG = NB // GRP
            NQ = 4                       # x DMA chunks per tile

            # Software pipeline: the 32 av matmuls + bv matmul for tile t-1
            # are emitted AFTER tile t's u-chain, so the PE never idles
            # waiting for the DVE un-copy at a tile boundary.
            xts = [None] * MT            # live xt tiles (av reads them late)
            uns = [None] * MT

            def av_block(t, last):
                nc.tensor.matmul(bv_ps[:], uns[t][:], uns[t][:],
                                 start=(t == 0), stop=last,
                                 skip_group_check=True)
                av_ps = pspool.tile([P, NB, R], F32, tag="avps")
                for nb in range(NB):
                    nc.tensor.matmul(av_ps[:, nb, :],
                                     xts[t][:, nb * P:(nb + 1) * P],
                                     uns[t][:], start=True, stop=True)
                if t == 0:
                    nc.vector.tensor_copy(av_acc[:], av_ps[:])
                else:
                    nc.vector.tensor_add(av_acc[:], av_acc[:], av_ps[:])

            for t in range(MT * repeat):
                rep, t = divmod(t, MT)
                xt = xpool.tile([P, N], F32, tag="xt")
                xts[t] = xt
                # all x chunks on the SP queue (its only job, 5.04us/tile of
                # issue time vs the 5.83us transfer budget); quarter-chunks
                # so the first transpose group unblocks at the 512KB mark
                for q in range(NQ):
                    lo, hi = q * (N // NQ), (q + 1) * (N // NQ)
                    nc.sync.dma_start(xt[:, lo:hi], xs_r[t][:, lo:hi])
                u_ps = pspool.tile([P, R], F32, tag="ups")

                # Group-level software pipeline: u-matmuls for group g-1 are
                # emitted after the transposes of group g, so the PE never
                # waits on the PSUM->SBUF copy of the group it just
                # transposed (the copy overlaps the next group instead).
                xTs = [None] * NG

                def u_mms(g):
                    for j in range(GRP):
                        nb = g * GRP + j
                        nc.tensor.matmul(u_ps[:], xTs[g][:, j, :],
                                         vw_sb[:, nb, :],
                                         start=(nb == 0), stop=False)

                for g in range(NG):
                    tp = ps3pool.tile([P, GRP, P], F32, tag="tp")
                    for j in range(GRP):
                        nb = g * GRP + j
                        nc.tensor.transpose(tp[:, j, :],
                                            xt[:, nb * P:(nb + 1) * P]
                                            .bitcast(F32R),
                                            id_sb[:].bitcast(F32R))
                    xT = xtpool.tile([P, GRP, P], F32, tag="xT")
                    xTs[g] = xT
                    if g % 2 == 1:
                        nc.scalar.copy(xT[:], tp[:])
                    else:
                        nc.vector.tensor_copy(xT[:], tp[:])
                    if g > 0:
                        u_mms(g - 1)
                u_mms(NG - 1)
                # u_old linear term + eps constant row
                nc.tensor.matmul(u_ps[:], uaug[:, t * P:(t + 1) * P],
                                 wa_sb[:], start=False, stop=True)
                un = spool.tile([P, R], F32, tag="un")
                uns[t] = un
                nc.vector.tensor_copy(un[:], u_ps[:])
                if t > 0:
                    # one tile late so un(t-1) is long done: the entry never
                    # blocks the Act queue behind it
                    nc.scalar.dma_start(uo_r[t - 1], uns[t - 1][:])
                    av_block(t - 1, last=False)
            nc.scalar.dma_start(uo_r[MT - 1], uns[MT - 1][:])
            av_block(MT - 1, last=True)

            nc.scalar.dma_start(av_r, av_acc[:])
            bv_sb = cpool.tile([R, R], F32)
            nc.vector.tensor_copy(bv_sb[:], bv_ps[:])
            nc.scalar.dma_start(bv_d[:], bv_sb[:])

    nc.compile()
    return nc


def _build_launch2():
    nc = bacc.Bacc("TRN2", target_bir_lowering=False, debug=False,
                   num_devices=NCORES)

    # aaug ([2R+1, NS]) and wcat ([2R+1, R]) fused into one input so a
    # single DMA covers both; v_out packed [P, NT*R] (host unpacks).
    ai_d = nc.dram_tensor("ain", [2 * R + 1, NS + R], F32,
                          kind="ExternalInput")
    vo_d = nc.dram_tensor("v_out", [P, NT * R], F32, kind="ExternalOutput")

    vo_r = vo_d[:].rearrange("p (t r) -> p t r", r=R)

    G2 = 4                                  # tiles per PSUM batch
    with tile.TileContext(nc) as tc:
        with (
            tc.tile_pool(name="sb", bufs=1) as pool,
            tc.tile_pool(name="ps", bufs=2, space="PSUM") as pspool,
        ):
            a_sb = pool.tile([2 * R + 1, NS + R], F32)
            nc.sync.dma_start(a_sb[:], ai_d[:])
            aa = a_sb[:, :NS]
            wc = a_sb[:, NS:]
            vn = pool.tile([P, NT, R], F32)
            for g in range(NT // G2):
                ps = pspool.tile([P, G2, R], F32, tag="vps")
                for j in range(G2):
                    t = g * G2 + j
                    nc.tensor.matmul(ps[:, j, :], aa[:, t * P:(t + 1) * P],
                                     wc, start=True, stop=True)
                dst = vn[:, g * G2:(g + 1) * G2, :]
                if g % 2 == 0:
                    nc.vector.tensor_copy(dst, ps[:])
                else:
                    nc.scalar.copy(dst, ps[:])
            nc.sync.dma_start(vo_r, vn[:])

    nc.compile()
    return nc


def _gs_coeffs(Bmat, eps=EPS):
    """Gauss-Seidel sweep as a linear map (float64).

    Returns W1, W3, c with u_new = a @ W1 - u_old @ W3 + c."""
    Rr = Bmat.shape[0]
    D = np.diag(np.diag(Bmat) + eps)
    W1 = np.linalg.inv(D + np.triu(Bmat, 1))
    W3 = np.tril(Bmat, -1) @ W1
    c = eps * W1.sum(axis=0)
    return W1, W3, c


LAST_EXEC_NS = None


def _run(nc, in_maps, trace=False):
    res = run_bass_kernel_spmd(nc, in_maps, list(range(NCORES)), trace=trace)
    return res


def _pack_rows(arr, p=P):
    """[T*p, r] row-major -> [p, T*r] partition-contiguous."""
    T = arr.shape[0] // p
    r = arr.shape[1]
    return np.ascontiguousarray(
        arr.reshape(T, p, r).transpose(1, 0, 2).reshape(p, T * r))


def _unpack_rows(arr, T, r, p=P):
    """[p, T*r] partition-contiguous -> [T*p, r] row-major."""
    return arr.reshape(p, T, r).transpose(1, 0, 2).reshape(T * p, r)


def kernel(x, u, v):
    global LAST_EXEC_NS
    x = np.ascontiguousarray(np.asarray(x, dtype=np.float32))
    u = np.ascontiguousarray(np.asarray(u, dtype=np.float32))
    v = np.ascontiguousarray(np.asarray(v, dtype=np.float32))

    if "l1" not in _cache:
        _cache["l1"] = _build_launch1()
    if "l2" not in _cache:
        _cache["l2"] = _build_launch2()

    import os
    trace = bool(os.environ.get("KERNEL_TRACE"))

    ident = np.eye(P, dtype=np.float32)

    # Host prep: u-side GS coefficients from v (R x R, float64)
    vw_all, wa_all = [], []
    for b in range(B):
        v64 = v[b].astype(np.float64)
        Bu = v64.T @ v64
        W1, W3, c = _gs_coeffs(Bu)
        vw_all.append(_pack_rows((v64 @ W1).astype(np.float32)))
        wa_all.append(np.concatenate([-W3, c[None, :]], axis=0)
                      .astype(np.float32))

    in_maps = []
    for core in range(NCORES):
        b, h = divmod(core, 2)
        ust = np.empty((R + 1, MS), dtype=np.float32)
        ust[:R] = u[b, h * MS:(h + 1) * MS, :].T
        ust[R] = 1.0
        in_maps.append({
            "xs": x[b, h * MS:(h + 1) * MS, :],
            "vw": vw_all[b],
            "ust": ust,
            "waug": wa_all[b],
            "ident": ident,
        })
    res1 = _run(_cache["l1"], in_maps, trace=trace)

    u_new = np.empty((B, M, R), dtype=np.float32)
    av = np.empty((B, N, R), dtype=np.float64)
    bv = np.empty((B, R, R), dtype=np.float64)
    for b in range(B):
        r0, r1 = res1.results[2 * b], res1.results[2 * b + 1]
        u_new[b, :MS] = r0["u_out"]
        u_new[b, MS:] = r1["u_out"]
        av[b] = (_unpack_rows(r0["av_out"], NB, R).astype(np.float64)
                 + _unpack_rows(r1["av_out"], NB, R).astype(np.float64))
        bv[b] = r0["bv_out"].astype(np.float64) + r1["bv_out"].astype(np.float64)

    # Host prep: v-side GS coefficients from device-computed B_v partials
    in_maps2 = []
    aaug = np.empty((B, 2 * R + 1, N), dtype=np.float32)
    wcat = np.empty((B, 2 * R + 1, R), dtype=np.float32)
    for b in range(B):
        W1v, W3v, cv = _gs_coeffs(bv[b])
        aaug[b, :R] = av[b].T
        aaug[b, R:2 * R] = v[b].T
        aaug[b, 2 * R] = 1.0
        wcat[b] = np.concatenate([W1v, -W3v, cv[None, :]], axis=0)
    for core in range(NCORES):
        b, h = divmod(core, 2)
        in_maps2.append({
            "ain": np.ascontiguousarray(
                np.concatenate([aaug[b, :, h * NS:(h + 1) * NS], wcat[b]],
                               axis=1)),
        })
    res2 = _run(_cache["l2"], in_maps2, trace=trace)

    v_new = np.empty((B, N, R), dtype=np.float32)
    for b in range(B):
        v_new[b, :NS] = _unpack_rows(res2.results[2 * b]["v_out"], NT, R)
        v_new[b, NS:] = _unpack_rows(res2.results[2 * b + 1]["v_out"], NT, R)

    t1 = res1.exec_time_ns
    t2 = res2.exec_time_ns
    LAST_EXEC_NS = (t1 or 0) + (t2 or 0) if (t1 or t2) else None

    return (u_new, v_new)
